# revision 11
# baseline (speedup 1.0000x reference)
"""Trainium2 Bass kernel: GAT(4-head) -> GAT -> 2x transformer encoder -> GAT head.

Self-contained: hardcodes the full-size problem config; distributes across
8 NeuronCores internally (node/graph parallel per sharding hint).

Strategy:
- Host: add self-loops, renumber nodes by descending in-degree, deal
  128-node blocks round-robin across cores. Edge slots live on a
  [128 part x T_tot] grid where partition row == dst-local row, so the
  per-dst aggregation is a plain free-axis reduce (no one-hot matmuls)
  and al_dst[dst] is a resident per-block slice.
- Device: phase A (x@W1 + logit tables) -> AllGather -> phase B (GAT1
  message pass + @W2 table) -> AllGather -> phase C (GAT2) -> phase T
  (transformer; seq_len=1 so attention == x@ (Wv@Wo) + b) -> AllGather ->
  phase D (GAT head).
"""
import math
import sys

sys.path.insert(0, '/opt/trn_rl_repo')
import numpy as np

import concourse.bass as bass
import concourse.mybir as mybir
import concourse.tile as tile
from concourse import bass_utils
from concourse.vector_clock import ScopedClock, VectorClock

F32 = mybir.dt.float32
I32 = mybir.dt.int32
AX = mybir.AxisListType.X
OP = mybir.AluOpType
ACTF = mybir.ActivationFunctionType
P = 128
LN_EPS = 1e-5
SM_EPS = 1e-16
NEG = -10000.0  # logit mask for padded edge slots


# ---------------------------------------------------------------------------
# walrus in this container rejects >1 sync-wait on the Tile tail drain;
# split the drain into one instruction per waited proc.
def _split_drain_and_barrier(self, tick_clock, wait_clock):
    gc = tick_clock.global_clock
    n = len(gc)
    nonzero = [(i, gc[i]) for i in range(n) if gc[i] > 0]
    chunks = [nonzero[i:i + 1] for i in range(len(nonzero))] or [[]]
    for chunk in chunks:
        vec = [0] * n
        for i, t in chunk:
            vec[i] = t
        drain_inst = self.nc.sync.drain()
        wait_clock.add_sem_waits(
            drain_inst.ins, ScopedClock({None: VectorClock(vec)}))
    self.nc.all_engine_barrier()
    assert self.sems is not None
    popped = self.nc._tile_sem_poison_stack.pop()
    assert popped is self._sem_poison
    self.nc.clear_and_free_semaphores(list(self.sems.allocated().values()))
    self.nc.all_engine_barrier()


tile.TileContext._drain_and_barrier = _split_drain_and_barrier


def split_excess_waits(nc, limit=1):
    """Hoist excess per-instruction sem waits onto same-engine NoOps --
    the container's walrus accepts only `limit` sync-wait commands/inst."""
    n_split = 0
    for f in nc.m.functions:
        for bb in f.blocks:
            new = []
            dirty = False
            for ins in bb.instructions:
                si = getattr(ins, 'sync_info', None)
                if si is not None and len(si.on_wait) > limit:
                    waits = list(si.on_wait)
                    excess, keep = waits[:-limit], waits[-limit:]
                    for i in range(0, len(excess), limit):
                        nop = mybir.InstNoOp(
                            name=f"{ins.name}_ws{i}",
                            sync_info=mybir.SyncInfo(
                                on_wait=excess[i:i + limit], on_update=[]),
                            bass_nofuse=True,
                            engine=ins.engine)
                        new.append(nop)
                    ins.sync_info = mybir.SyncInfo(
                        on_wait=keep, on_update=list(si.on_update))
                    dirty = True
                    n_split += 1
                new.append(ins)
            if dirty:
                bb.instructions = new
    return n_split
# ---------------------------------------------------------------------------


def make_cfg(N, E, IN_DIM, HID, HEADS, OUT, FF, NLAYERS, NCORES=8,
             CH1=16, CH2=24, CH3=48, S=4):
    c = {}
    c['N'], c['E'], c['IN_DIM'], c['HID'] = N, E, IN_DIM, HID
    c['HEADS'], c['OUT'], c['FF'], c['NLAYERS'] = HEADS, OUT, FF, NLAYERS
    c['NCORES'] = NCORES
    c['KCH'] = (IN_DIM + P - 1) // P
    c['INP'] = c['KCH'] * P
    gblk = (N + P - 1) // P
    c['NBLK'] = (gblk + NCORES - 1) // NCORES
    c['Npad'] = c['NBLK'] * NCORES * P
    c['NSC'] = c['NBLK'] * P
    c['HW1'] = HEADS * HID
    c['N2CH'] = (c['HW1'] + P - 1) // P
    c['FFCH'] = (FF + P - 1) // P
    c['FFP'] = c['FFCH'] * P
    c['T1C'] = c['HW1'] + HEADS
    c['T2C'] = HID + 1
    c['T3C'] = OUT + 1
    c['CH1'], c['CH2'], c['CH3'], c['S'] = CH1, CH2, CH3, S
    return c


def _bc(row, n=P):
    """host: broadcast a [D] row to [n, D] float32."""
    return np.ascontiguousarray(
        np.broadcast_to(np.asarray(row, np.float32), (n, len(row))))


def host_prepare(cfg, inp):
    N, E = cfg['N'], cfg['E']
    NCORES, NBLK, NSC, Npad = (cfg['NCORES'], cfg['NBLK'], cfg['NSC'],
                               cfg['Npad'])
    IN_DIM, INP, KCH = cfg['IN_DIM'], cfg['INP'], cfg['KCH']
    HID, HEADS, OUT, FF, L = (cfg['HID'], cfg['HEADS'], cfg['OUT'],
                              cfg['FF'], cfg['NLAYERS'])
    HW1, N2CH, FFCH, FFP = cfg['HW1'], cfg['N2CH'], cfg['FFCH'], cfg['FFP']

    src = np.concatenate([np.asarray(inp['edge_src']),
                          np.arange(N, dtype=np.int32)]).astype(np.int64)
    dst = np.concatenate([np.asarray(inp['edge_dst']),
                          np.arange(N, dtype=np.int32)]).astype(np.int64)
    deg = np.bincount(dst, minlength=N)
    perm = np.argsort(-deg, kind='stable')          # new -> old
    invp = np.empty(N, np.int64)
    invp[perm] = np.arange(N)
    s2, d2 = invp[src], invp[dst]
    degs = deg[perm]                                # descending

    order = np.argsort(d2, kind='stable')
    s2s, d2s = s2[order], d2[order]
    counts = np.bincount(d2s, minlength=Npad)
    starts = np.concatenate([[0], np.cumsum(counts)[:-1]])
    tpos = np.arange(len(d2s)) - starts[d2s]

    g = d2s // P
    prow = d2s % P
    core = g % NCORES
    jblk = g // NCORES

    TPB = []
    for j in range(NBLK):
        n0 = j * NCORES * P
        TPB.append(int(degs[n0]) if n0 < N else 1)
        TPB[-1] = max(1, TPB[-1])
    offs = np.concatenate([[0], np.cumsum(TPB)]).astype(np.int64)
    T_tot = int(offs[-1])

    gs = s2s // P
    tablerow = (gs % NCORES) * NSC + (gs // NCORES) * P + (s2s % P)

    srcidx = np.zeros((NCORES, P, T_tot), np.int32)
    logmask = np.full((NCORES, P, T_tot), NEG, np.float32)
    col = offs[jblk] + tpos
    srcidx[core, prow, col] = tablerow.astype(np.int32)
    logmask[core, prow, col] = 0.0

    x = np.asarray(inp['x'], np.float32)
    xT = np.zeros((NCORES, INP, NSC), np.float32)
    node_new = {}
    for cc in range(NCORES):
        gids = np.arange(NBLK) * NCORES + cc
        nn = (gids[:, None] * P + np.arange(P)[None, :]).reshape(-1)
        node_new[cc] = nn
        valid = nn < N
        xr = np.zeros((NSC, IN_DIM), np.float32)
        xr[valid] = x[perm[nn[valid]]]
        xT[cc, :IN_DIM, :] = xr.T

    # weights (shared by all cores)
    W1 = np.zeros((KCH * P, HW1), np.float32)
    W1[:IN_DIM] = np.asarray(inp['W1'], np.float32)
    W1c = np.ascontiguousarray(
        W1.reshape(KCH, P, HW1).transpose(1, 0, 2).reshape(P, KCH * HW1))
    W2 = np.zeros((N2CH * P, HID), np.float32)
    W2[:HW1] = np.asarray(inp['W2'], np.float32)
    W2c = np.ascontiguousarray(
        W2.reshape(N2CH, P, HID).transpose(1, 0, 2).reshape(P, N2CH * HID))

    com = {
        'i_W1c': W1c,
        'i_asx1': _bc(np.asarray(inp['a_src1'], np.float32).reshape(-1)),
        'i_adx1': _bc(np.asarray(inp['a_dst1'], np.float32).reshape(-1)),
        'i_b1x': _bc(np.asarray(inp['b1'], np.float32)),
        'i_W2c': W2c,
        'i_asx2': _bc(np.asarray(inp['a_src2'], np.float32).reshape(-1)),
        'i_adx2': _bc(np.asarray(inp['a_dst2'], np.float32).reshape(-1)),
        'i_b2x': _bc(np.asarray(inp['b2'], np.float32)),
        'i_W3': np.asarray(inp['W3'], np.float32),
        'i_a3sx': _bc(np.asarray(inp['a_src3'], np.float32).reshape(-1)),
        'i_a3dx': _bc(np.asarray(inp['a_dst3'], np.float32).reshape(-1)),
        'i_b3x': _bc(np.asarray(inp['b3'], np.float32)),
        'i_ident': np.eye(P, dtype=np.float32),
    }
    Wvo = np.zeros((L, HID, HID), np.float32)
    bvox = np.zeros((L, P, HID), np.float32)
    Wf1 = np.zeros((L, HID, FFP), np.float32)
    bf1c = np.zeros((L, P, FFCH), np.float32)
    Wf2c = np.zeros((L, P, FFCH * HID), np.float32)
    for l in range(L):
        wv = np.asarray(inp['t_Wv'][l], np.float32)
        wo = np.asarray(inp['t_Wo'][l], np.float32)
        Wvo[l] = wv @ wo
        bvox[l] = _bc(np.asarray(inp['t_bv'][l], np.float32) @ wo
                      + np.asarray(inp['t_bo'][l], np.float32))
        Wf1[l, :, :FF] = np.asarray(inp['t_Wff1'][l], np.float32)
        bf1 = np.zeros(FFP, np.float32)
        bf1[:FF] = np.asarray(inp['t_bff1'][l], np.float32)
        bf1c[l] = bf1.reshape(FFCH, P).T
        wf2 = np.zeros((FFP, HID), np.float32)
        wf2[:FF] = np.asarray(inp['t_Wff2'][l], np.float32)
        Wf2c[l] = wf2.reshape(FFCH, P, HID).transpose(1, 0, 2).reshape(
            P, FFCH * HID)
    com['i_Wvo'] = Wvo
    com['i_bvox'] = bvox
    com['i_Wf1'] = Wf1
    com['i_bf1c'] = bf1c
    com['i_Wf2c'] = Wf2c
    com['i_g1x'] = np.stack(
        [_bc(np.asarray(inp['t_ln1g'][l], np.float32)) for l in range(L)])
    com['i_b1lx'] = np.stack(
        [_bc(np.asarray(inp['t_ln1b'][l], np.float32)) for l in range(L)])
    com['i_g2x'] = np.stack(
        [_bc(np.asarray(inp['t_ln2g'][l], np.float32)) for l in range(L)])
    com['i_b2lx'] = np.stack(
        [_bc(np.asarray(inp['t_ln2b'][l], np.float32)) for l in range(L)])
    com['i_bf2x'] = np.stack(
        [_bc(np.asarray(inp['t_bff2'][l], np.float32)) for l in range(L)])

    in_maps = []
    for cc in range(NCORES):
        m = dict(com)
        m['i_xT'] = xT[cc]
        m['i_srcidx'] = srcidx[cc]
        m['i_logmask'] = logmask[cc]
        in_maps.append(m)

    meta = {'TPB': TPB, 'offs': offs, 'T_tot': T_tot}

    def post(results):
        fused = np.empty((N, HID), np.float32)
        adt = np.empty((N, OUT), np.float32)
        for cc in range(NCORES):
            nn = node_new[cc]
            valid = nn < N
            old = perm[nn[valid]]
            fused[old] = results[cc]['o_fused'][valid]
            adt[old] = results[cc]['o_adt'][valid]
        return adt, fused

    return in_maps, meta, post


def _ln(nc, sp, xv, Sg, HID, gx, bx, tag, epsln):
    """LayerNorm over the last (HID) axis of xv = [P, Sg, HID] view, in place."""
    mu = sp.tile([P, Sg], F32, tag=tag + "mu", name=tag + "mu")
    nc.vector.reduce_sum(out=mu[:], in_=xv, axis=AX)
    nc.vector.tensor_scalar_mul(out=mu[:], in0=mu[:], scalar1=1.0 / HID)
    nc.vector.tensor_tensor(
        out=xv, in0=xv, in1=mu[:].unsqueeze(2).to_broadcast([P, Sg, HID]),
        op=OP.subtract)
    sq = sp.tile([P, Sg * HID], F32, tag=tag + "sq", name=tag + "sq")
    nc.scalar.activation(out=sq[:], in_=xv, func=ACTF.Square)
    var = sp.tile([P, Sg], F32, tag=tag + "var", name=tag + "var")
    nc.vector.reduce_sum(
        out=var[:], in_=sq[:].rearrange("p (s d) -> p s d", s=Sg), axis=AX)
    nc.scalar.activation(out=var[:], in_=var[:], func=ACTF.Sqrt,
                         bias=epsln[:], scale=1.0 / HID)
    nc.vector.reciprocal(out=var[:], in_=var[:])
    nc.vector.tensor_tensor(
        out=xv, in0=xv, in1=var[:].unsqueeze(2).to_broadcast([P, Sg, HID]),
        op=OP.mult)
    nc.vector.tensor_tensor(
        out=xv, in0=xv,
        in1=gx[:].unsqueeze(1).to_broadcast([P, Sg, HID]), op=OP.mult)
    nc.vector.tensor_tensor(
        out=xv, in0=xv,
        in1=bx[:].unsqueeze(1).to_broadcast([P, Sg, HID]), op=OP.add)


def _elu_into(nc, sp, F, out_ap, width, tag):
    """out = elu(F) for [P, width] tile F (F is clobbered)."""
    te = sp.tile([P, width], F32, tag=tag, name=tag)
    nc.vector.tensor_scalar_min(out=te[:], in0=F[:], scalar1=0.0)
    nc.scalar.activation(out=te[:], in_=te[:], func=ACTF.Exp)
    nc.vector.tensor_scalar_add(out=te[:], in0=te[:], scalar1=-1.0)
    nc.vector.tensor_scalar_max(out=F[:], in0=F[:], scalar1=0.0)
    nc.vector.tensor_tensor(out=out_ap, in0=F[:], in1=te[:], op=OP.add)


def build_program(cfg, meta):
    NCORES, NBLK, NSC, Npad = (cfg['NCORES'], cfg['NBLK'], cfg['NSC'],
                               cfg['Npad'])
    KCH, HID, HEADS, OUT, L = (cfg['KCH'], cfg['HID'], cfg['HEADS'],
                               cfg['OUT'], cfg['NLAYERS'])
    HW1, N2CH, FFCH, FFP = cfg['HW1'], cfg['N2CH'], cfg['FFCH'], cfg['FFP']
    T1C, T2C, T3C = cfg['T1C'], cfg['T2C'], cfg['T3C']
    CH1, CH2, CH3, S = cfg['CH1'], cfg['CH2'], cfg['CH3'], cfg['S']
    TPB, offs, T_tot = meta['TPB'], meta['offs'], meta['T_tot']
    RG = [list(range(NCORES))]

    nc = bass.Bass("TRN2", target_bir_lowering=False, debug=False,
                   num_devices=NCORES)
    t_xT = nc.dram_tensor("i_xT", [cfg['INP'], NSC], F32, kind="ExternalInput")
    t_src = nc.dram_tensor("i_srcidx", [P, T_tot], I32, kind="ExternalInput")
    t_lm = nc.dram_tensor("i_logmask", [P, T_tot], F32, kind="ExternalInput")
    t_W1c = nc.dram_tensor("i_W1c", [P, KCH * HW1], F32, kind="ExternalInput")
    t_asx1 = nc.dram_tensor("i_asx1", [P, HW1], F32, kind="ExternalInput")
    t_adx1 = nc.dram_tensor("i_adx1", [P, HW1], F32, kind="ExternalInput")
    t_b1x = nc.dram_tensor("i_b1x", [P, HW1], F32, kind="ExternalInput")
    t_W2c = nc.dram_tensor("i_W2c", [P, N2CH * HID], F32, kind="ExternalInput")
    t_asx2 = nc.dram_tensor("i_asx2", [P, HID], F32, kind="ExternalInput")
    t_adx2 = nc.dram_tensor("i_adx2", [P, HID], F32, kind="ExternalInput")
    t_b2x = nc.dram_tensor("i_b2x", [P, HID], F32, kind="ExternalInput")
    t_Wvo = nc.dram_tensor("i_Wvo", [L, HID, HID], F32, kind="ExternalInput")
    t_bvox = nc.dram_tensor("i_bvox", [L, P, HID], F32, kind="ExternalInput")
    t_Wf1 = nc.dram_tensor("i_Wf1", [L, HID, FFP], F32, kind="ExternalInput")
    t_bf1c = nc.dram_tensor("i_bf1c", [L, P, FFCH], F32, kind="ExternalInput")
    t_Wf2c = nc.dram_tensor("i_Wf2c", [L, P, FFCH * HID], F32,
                            kind="ExternalInput")
    t_bf2x = nc.dram_tensor("i_bf2x", [L, P, HID], F32, kind="ExternalInput")
    t_g1x = nc.dram_tensor("i_g1x", [L, P, HID], F32, kind="ExternalInput")
    t_b1lx = nc.dram_tensor("i_b1lx", [L, P, HID], F32, kind="ExternalInput")
    t_g2x = nc.dram_tensor("i_g2x", [L, P, HID], F32, kind="ExternalInput")
    t_b2lx = nc.dram_tensor("i_b2lx", [L, P, HID], F32, kind="ExternalInput")
    t_W3 = nc.dram_tensor("i_W3", [HID, OUT], F32, kind="ExternalInput")
    t_a3sx = nc.dram_tensor("i_a3sx", [P, OUT], F32, kind="ExternalInput")
    t_a3dx = nc.dram_tensor("i_a3dx", [P, OUT], F32, kind="ExternalInput")
    t_b3x = nc.dram_tensor("i_b3x", [P, OUT], F32, kind="ExternalInput")
    t_ident = nc.dram_tensor("i_ident", [P, P], F32, kind="ExternalInput")
    t_fused = nc.dram_tensor("o_fused", [NSC, HID], F32, kind="ExternalOutput")
    t_adt = nc.dram_tensor("o_adt", [NSC, OUT], F32, kind="ExternalOutput")
    DBG = cfg.get('DBG', False)
    if DBG:
        t_dt1 = nc.dram_tensor("o_t1", [Npad, T1C], F32, kind="ExternalOutput")
        t_dt2 = nc.dram_tensor("o_t2", [Npad, T2C], F32, kind="ExternalOutput")
        t_dt3 = nc.dram_tensor("o_t3", [Npad, T3C], F32, kind="ExternalOutput")
        t_drna = nc.dram_tensor("o_rna", [P, NBLK * HID], F32, kind="ExternalOutput")
        t_dad1 = nc.dram_tensor("o_ad1", [P, NBLK * HEADS], F32, kind="ExternalOutput")
        t_dG0 = nc.dram_tensor("o_G0", [P, CH1 * T1C], F32, kind="ExternalOutput")
        t_dS1 = nc.dram_tensor("o_S1", [P, NBLK * T1C], F32, kind="ExternalOutput")
        t_dh1f = nc.dram_tensor("o_h1f", [P, NBLK * HW1], F32, kind="ExternalOutput")

    with tile.TileContext(nc) as tc:
        cp = tc.alloc_tile_pool(name="cp", bufs=1)
        dp = tc.alloc_tile_pool(name="dp", bufs=1, space="DRAM")

        def ld(t, shape, dtype=F32, nm=None):
            tl = cp.tile(shape, dtype, name=nm or ("c_" + t.name))
            nc.sync.dma_start(out=tl[:], in_=t.ap())
            return tl

        W1c = ld(t_W1c, [P, KCH * HW1])
        asx1 = ld(t_asx1, [P, HW1])
        adx1 = ld(t_adx1, [P, HW1])
        b1x = ld(t_b1x, [P, HW1])
        W2c = ld(t_W2c, [P, N2CH * HID])
        asx2 = ld(t_asx2, [P, HID])
        adx2 = ld(t_adx2, [P, HID])
        b2x = ld(t_b2x, [P, HID])
        W3s = ld(t_W3, [HID, OUT])
        a3sx = ld(t_a3sx, [P, OUT])
        a3dx = ld(t_a3dx, [P, OUT])
        b3x = ld(t_b3x, [P, OUT])
        ident = ld(t_ident, [P, P])
        srci = ld(t_src, [P, T_tot], I32)
        lmsk = ld(t_lm, [P, T_tot])
        Wvo, bvox, Wf1, bf1c, Wf2c, bf2x, g1x, b1lx, g2x, b2lx = (
            [], [], [], [], [], [], [], [], [], [])
        for l in range(L):
            for lst, t, shape in (
                    (Wvo, t_Wvo, [HID, HID]), (bvox, t_bvox, [P, HID]),
                    (Wf1, t_Wf1, [HID, FFP]), (bf1c, t_bf1c, [P, FFCH]),
                    (Wf2c, t_Wf2c, [P, FFCH * HID]), (bf2x, t_bf2x, [P, HID]),
                    (g1x, t_g1x, [P, HID]), (b1lx, t_b1lx, [P, HID]),
                    (g2x, t_g2x, [P, HID]), (b2lx, t_b2lx, [P, HID])):
                tl = cp.tile(shape, F32, name=f"c_{t.name}_{l}")
                nc.sync.dma_start(out=tl[:], in_=t.ap()[l])
                lst.append(tl)

        epsln = cp.tile([P, 1], F32, name="epsln")
        nc.vector.memset(epsln[:], LN_EPS)
        adL1 = cp.tile([P, NBLK * HEADS], F32, name="adL1")
        adL2 = cp.tile([P, NBLK], F32, name="adL2")
        adL3 = cp.tile([P, NBLK], F32, name="adL3")
        rna = cp.tile([P, NBLK * HID], F32, name="rna")
        ag1_in = dp.tile([NSC, T1C], F32, name="ag1_in")
        table1 = dp.tile([Npad, T1C], F32, addr_space="Shared", name="table1")
        ag2_in = dp.tile([NSC, T2C], F32, name="ag2_in")
        table2 = dp.tile([Npad, T2C], F32, addr_space="Shared", name="table2")
        ag3_in = dp.tile([NSC, T3C], F32, name="ag3_in")
        table3 = dp.tile([Npad, T3C], F32, addr_space="Shared", name="table3")

        # ---------------- PHASE A: h1 = x @ W1, logit tables ---------------
        with tc.tile_pool(name="pa", bufs=3) as sp, \
             tc.tile_pool(name="pap", bufs=2, space="PSUM") as pp:
            xTr = t_xT.ap().rearrange("(k p) n -> p k n", p=P)
            for j in range(NBLK):
                xblk = sp.tile([P, KCH * P], F32, tag="xblk", name="xblk")
                nc.sync.dma_start(
                    out=xblk[:].rearrange("p (k m) -> p k m", k=KCH),
                    in_=xTr[:, :, j * P:(j + 1) * P])
                ph = pp.tile([P, HW1], F32, tag="ph", name="ph")
                for kk in range(KCH):
                    nc.tensor.matmul(
                        ph[:], lhsT=xblk[:, kk * P:(kk + 1) * P],
                        rhs=W1c[:, kk * HW1:(kk + 1) * HW1],
                        start=(kk == 0), stop=(kk == KCH - 1))
                hrow = sp.tile([P, T1C], F32, tag="hrow", name="hrow")
                nc.vector.tensor_copy(out=hrow[:, 0:HW1], in_=ph[:])
                tmp = sp.tile([P, HW1], F32, tag="tmpa", name="tmpa")
                nc.vector.tensor_tensor(out=tmp[:], in0=hrow[:, 0:HW1],
                                        in1=asx1[:], op=OP.mult)
                nc.vector.reduce_sum(
                    out=hrow[:, HW1:HW1 + HEADS],
                    in_=tmp[:].rearrange("p (h d) -> p h d", h=HEADS), axis=AX)
                nc.vector.tensor_tensor(out=tmp[:], in0=hrow[:, 0:HW1],
                                        in1=adx1[:], op=OP.mult)
                nc.vector.reduce_sum(
                    out=adL1[:, j * HEADS:(j + 1) * HEADS],
                    in_=tmp[:].rearrange("p (h d) -> p h d", h=HEADS), axis=AX)
                nc.sync.dma_start(out=ag1_in[j * P:(j + 1) * P, :],
                                  in_=hrow[:])
        nc.gpsimd.collective_compute(
            "AllGather", OP.bypass, replica_groups=RG,
            ins=[ag1_in[:]], outs=[table1[:]])

        # -------- PHASE B: GAT1 message passing, then h2 = elu() @ W2 ------
        with tc.tile_pool(name="pb", bufs=2) as sp, \
             tc.tile_pool(name="pbs", bufs=2) as sps, \
             tc.tile_pool(name="pbp", bufs=2, space="PSUM") as pp:
            for j in range(NBLK):
                nchk = (TPB[j] + CH1 - 1) // CH1
                S1 = sps.tile([P, T1C], F32, tag="S1", name="S1")
                for ci in range(nchk):
                    c0 = ci * CH1
                    cw = min(CH1, TPB[j] - c0)
                    G = sp.tile([P, CH1 * T1C], F32, tag="G1", name="G1")
                    M = sp.tile([P, CH1 * T1C], F32, tag="M1", name="M1")
                    Gv = G[:, :cw * T1C].rearrange("p (c w) -> p c w", w=T1C)
                    Mv = M[:, :cw * T1C].rearrange("p (c w) -> p c w", w=T1C)
                    for t in range(cw):
                        nc.gpsimd.indirect_dma_start(
                            out=G[:, t * T1C:(t + 1) * T1C], out_offset=None,
                            in_=table1[:],
                            in_offset=bass.IndirectOffsetOnAxis(
                                ap=srci[:, offs[j] + c0 + t:
                                        offs[j] + c0 + t + 1], axis=0))
                    if DBG and j == 0 and ci == 0:
                        nc.sync.dma_start(out=t_dG0.ap()[:, :cw * T1C],
                                          in_=G[:, :cw * T1C])
                    nc.vector.tensor_tensor(
                        out=Mv[:, :, HW1:], in0=Gv[:, :, HW1:],
                        in1=adL1[:, j * HEADS:(j + 1) * HEADS].unsqueeze(1)
                        .to_broadcast([P, cw, HEADS]), op=OP.add)
                    nc.vector.tensor_tensor(
                        out=Mv[:, :, HW1:], in0=Mv[:, :, HW1:],
                        in1=lmsk[:, offs[j] + c0: offs[j] + c0 + cw]
                        .unsqueeze(2).to_broadcast([P, cw, HEADS]), op=OP.add)
                    zt = sp.tile([P, CH1 * HEADS], F32, tag="zt1",
                                 name="zt1")
                    zv = zt[:, :cw * HEADS].rearrange(
                        "p (c h) -> p c h", h=HEADS)
                    nc.vector.tensor_scalar_mul(out=zv, in0=Mv[:, :, HW1:],
                                                scalar1=0.2)
                    nc.vector.tensor_tensor(out=Mv[:, :, HW1:],
                                            in0=Mv[:, :, HW1:], in1=zv,
                                            op=OP.max)
                    nc.scalar.activation(out=Mv[:, :, HW1:],
                                         in_=Mv[:, :, HW1:], func=ACTF.Exp)
                    for hd in range(HEADS):
                        nc.vector.tensor_tensor(
                            out=Mv[:, :, hd * HID:(hd + 1) * HID],
                            in0=Gv[:, :, hd * HID:(hd + 1) * HID],
                            in1=Mv[:, :, HW1 + hd:HW1 + hd + 1]
                            .to_broadcast([P, cw, HID]), op=OP.mult)
                    Mt = M[:, :cw * T1C].rearrange(
                        "p (c w) -> p c w", w=T1C).transpose([0, 2, 1])
                    if ci == 0:
                        nc.vector.reduce_sum(out=S1[:], in_=Mt, axis=AX)
                    else:
                        St = sp.tile([P, T1C], F32, tag="St1", name="St1")
                        nc.vector.reduce_sum(out=St[:], in_=Mt, axis=AX)
                        nc.vector.tensor_add(out=S1[:], in0=S1[:], in1=St[:])
                if DBG:
                    nc.sync.dma_start(
                        out=t_dS1.ap()[:, j * T1C:(j + 1) * T1C], in_=S1[:])
                rec = sps.tile([P, HEADS], F32, tag="rec1", name="rec1")
                nc.vector.tensor_scalar_add(out=rec[:], in0=S1[:, HW1:],
                                            scalar1=SM_EPS)
                nc.vector.reciprocal(out=rec[:], in_=rec[:])
                F = sps.tile([P, HW1], F32, tag="F1", name="F1")
                for hd in range(HEADS):
                    nc.vector.tensor_scalar_mul(
                        out=F[:, hd * HID:(hd + 1) * HID],
                        in0=S1[:, hd * HID:(hd + 1) * HID],
                        scalar1=rec[:, hd:hd + 1])
                nc.vector.tensor_tensor(out=F[:], in0=F[:], in1=b1x[:],
                                        op=OP.add)
                h1f = sps.tile([P, HW1], F32, tag="h1f", name="h1f")
                _elu_into(nc, sps, F, h1f[:], HW1, "elu1")
                if DBG:
                    nc.sync.dma_start(
                        out=t_dh1f.ap()[:, j * HW1:(j + 1) * HW1],
                        in_=h1f[:])
                hT = sps.tile([P, N2CH * P], F32, tag="hT", name="hT")
                for ck in range(N2CH):
                    cw2 = min(P, HW1 - ck * P)
                    pt = pp.tile([P, P], F32, tag="pt", name="pt")
                    nc.tensor.transpose(out=pt[:cw2, :],
                                        in_=h1f[:, ck * P:ck * P + cw2],
                                        identity=ident[:])
                    nc.vector.tensor_copy(out=hT[:cw2, ck * P:(ck + 1) * P],
                                          in_=pt[:cw2, :])
                p2 = pp.tile([P, HID], F32, tag="p2", name="p2")
                for ck in range(N2CH):
                    cw2 = min(P, HW1 - ck * P)
                    nc.tensor.matmul(
                        p2[:], lhsT=hT[:cw2, ck * P:(ck + 1) * P],
                        rhs=W2c[:cw2, ck * HID:(ck + 1) * HID],
                        start=(ck == 0), stop=(ck == N2CH - 1))
                t2r = sps.tile([P, T2C], F32, tag="t2r", name="t2r")
                nc.vector.tensor_copy(out=t2r[:, 0:HID], in_=p2[:])
                tmp2 = sps.tile([P, HID], F32, tag="tmp2", name="tmp2")
                nc.vector.tensor_tensor(out=tmp2[:], in0=t2r[:, :HID],
                                        in1=asx2[:], op=OP.mult)
                nc.vector.reduce_sum(out=t2r[:, HID:HID + 1], in_=tmp2[:],
                                     axis=AX)
                nc.vector.tensor_tensor(out=tmp2[:], in0=t2r[:, :HID],
                                        in1=adx2[:], op=OP.mult)
                nc.vector.reduce_sum(out=adL2[:, j:j + 1], in_=tmp2[:],
                                     axis=AX)
                nc.sync.dma_start(out=ag2_in[j * P:(j + 1) * P, :],
                                  in_=t2r[:])
        nc.gpsimd.collective_compute(
            "AllGather", OP.bypass, replica_groups=RG,
            ins=[ag2_in[:]], outs=[table2[:]])

        # ---------------- PHASE C: GAT2 -> rna (resident) -------------------
        with tc.tile_pool(name="pc", bufs=2) as sp, \
             tc.tile_pool(name="pcs", bufs=2) as sps:
            for j in range(NBLK):
                nchk = (TPB[j] + CH2 - 1) // CH2
                S2 = sps.tile([P, T2C], F32, tag="S2", name="S2")
                for ci in range(nchk):
                    c0 = ci * CH2
                    cw = min(CH2, TPB[j] - c0)
                    G = sp.tile([P, CH2 * T2C], F32, tag="G2", name="G2")
                    M = sp.tile([P, CH2 * T2C], F32, tag="M2", name="M2")
                    Gv = G[:, :cw * T2C].rearrange("p (c w) -> p c w", w=T2C)
                    Mv = M[:, :cw * T2C].rearrange("p (c w) -> p c w", w=T2C)
                    for t in range(cw):
                        nc.gpsimd.indirect_dma_start(
                            out=G[:, t * T2C:(t + 1) * T2C], out_offset=None,
                            in_=table2[:],
                            in_offset=bass.IndirectOffsetOnAxis(
                                ap=srci[:, offs[j] + c0 + t:
                                        offs[j] + c0 + t + 1], axis=0))
                    nc.vector.tensor_tensor(
                        out=Mv[:, :, HID:], in0=Gv[:, :, HID:],
                        in1=adL2[:, j:j + 1].unsqueeze(1)
                        .to_broadcast([P, cw, 1]), op=OP.add)
                    nc.vector.tensor_tensor(
                        out=Mv[:, :, HID:], in0=Mv[:, :, HID:],
                        in1=lmsk[:, offs[j] + c0: offs[j] + c0 + cw]
                        .unsqueeze(2).to_broadcast([P, cw, 1]), op=OP.add)
                    zt = sp.tile([P, CH2], F32, tag="zt2", name="zt2")
                    zv = zt[:, :cw].unsqueeze(2)
                    nc.vector.tensor_scalar_mul(out=zv, in0=Mv[:, :, HID:],
                                                scalar1=0.2)
                    nc.vector.tensor_tensor(out=Mv[:, :, HID:],
                                            in0=Mv[:, :, HID:], in1=zv,
                                            op=OP.max)
                    nc.scalar.activation(out=Mv[:, :, HID:],
                                         in_=Mv[:, :, HID:], func=ACTF.Exp)
                    nc.vector.tensor_tensor(
                        out=Mv[:, :, 0:HID], in0=Gv[:, :, 0:HID],
                        in1=Mv[:, :, HID:HID + 1].to_broadcast([P, cw, HID]),
                        op=OP.mult)
                    Mt = M[:, :cw * T2C].rearrange(
                        "p (c w) -> p c w", w=T2C).transpose([0, 2, 1])
                    if ci == 0:
                        nc.vector.reduce_sum(out=S2[:], in_=Mt, axis=AX)
                    else:
                        St = sp.tile([P, T2C], F32, tag="St2", name="St2")
                        nc.vector.reduce_sum(out=St[:], in_=Mt, axis=AX)
                        nc.vector.tensor_add(out=S2[:], in0=S2[:], in1=St[:])
                rec = sps.tile([P, 1], F32, tag="rec2", name="rec2")
                nc.vector.tensor_scalar_add(out=rec[:], in0=S2[:, HID:],
                                            scalar1=SM_EPS)
                nc.vector.reciprocal(out=rec[:], in_=rec[:])
                F = sps.tile([P, HID], F32, tag="F2", name="F2")
                nc.vector.tensor_scalar_mul(out=F[:], in0=S2[:, 0:HID],
                                            scalar1=rec[:, 0:1])
                nc.vector.tensor_tensor(out=F[:], in0=F[:], in1=b2x[:],
                                        op=OP.add)
                _elu_into(nc, sps, F, rna[:, j * HID:(j + 1) * HID], HID,
                          "elu2")
        # ---------------- PHASE T: transformer encoder x2 ------------------
        with tc.tile_pool(name="pt", bufs=2) as sp, \
             tc.tile_pool(name="ptp", bufs=2, space="PSUM") as pp:
            groups = [(g0, min(S, NBLK - g0)) for g0 in range(0, NBLK, S)]
            for (g0, Sg) in groups:
                W = Sg * HID
                xc = sp.tile([P, S * HID], F32, tag="xc", name="xc")
                nc.vector.tensor_copy(
                    out=xc[:, :W], in_=rna[:, g0 * HID:(g0 + Sg) * HID])
                for l in range(L):
                    xTg = sp.tile([HID, S * P], F32, tag="xTg", name="xTg")
                    for s in range(Sg):
                        ptt = pp.tile([HID, P], F32, tag="ptt", name="ptt", bufs=1)
                        nc.tensor.transpose(
                            out=ptt[:], in_=xc[:, (s * HID):(s + 1) * HID],
                            identity=ident[:])
                        nc.vector.tensor_copy(
                            out=xTg[:, s * P:(s + 1) * P], in_=ptt[:])
                    pa = pp.tile([P, S * HID], F32, tag="pa", name="pa", bufs=1)
                    for s in range(Sg):
                        nc.tensor.matmul(
                            pa[:, s * HID:(s + 1) * HID],
                            lhsT=xTg[:, s * P:(s + 1) * P], rhs=Wvo[l][:],
                            start=True, stop=True)
                    nc.vector.tensor_tensor(out=xc[:, :W], in0=xc[:, :W],
                                            in1=pa[:, :W], op=OP.add)
                    nc.vector.tensor_tensor(
                        out=xc[:, :W].rearrange("p (s d) -> p s d", s=Sg),
                        in0=xc[:, :W].rearrange("p (s d) -> p s d", s=Sg),
                        in1=bvox[l][:].unsqueeze(1).to_broadcast(
                            [P, Sg, HID]), op=OP.add)
                    _ln(nc, sp,
                        xc[:, :W].rearrange("p (s d) -> p s d", s=Sg),
                        Sg, HID, g1x[l], b1lx[l], "ln1", epsln)
                    x1Tg = sp.tile([HID, S * P], F32, tag="x1Tg", name="x1Tg")
                    for s in range(Sg):
                        ptt = pp.tile([HID, P], F32, tag="ptt", name="ptt", bufs=1)
                        nc.tensor.transpose(
                            out=ptt[:], in_=xc[:, (s * HID):(s + 1) * HID],
                            identity=ident[:])
                        nc.vector.tensor_copy(
                            out=x1Tg[:, s * P:(s + 1) * P], in_=ptt[:])
                    pffT = pp.tile([HID, S * P], F32, tag="pffT", name="pffT", bufs=1)
                    for kk in range(FFCH):
                        pf1 = pp.tile([P, S * P], F32, tag="pf1", name="pf1")
                        nc.tensor.matmul(
                            pf1[:, :Sg * P],
                            lhsT=Wf1[l][:, kk * P:(kk + 1) * P],
                            rhs=x1Tg[:, :Sg * P], start=True, stop=True)
                        rel = sp.tile([P, S * P], F32, tag="rel", name="rel")
                        nc.scalar.activation(
                            out=rel[:, :Sg * P], in_=pf1[:, :Sg * P],
                            func=ACTF.Relu, bias=bf1c[l][:, kk:kk + 1],
                            scale=1.0)
                        nc.tensor.matmul(
                            pffT[:, :Sg * P],
                            lhsT=Wf2c[l][:, kk * HID:(kk + 1) * HID],
                            rhs=rel[:, :Sg * P],
                            start=(kk == 0), stop=(kk == FFCH - 1))
                    ffT = sp.tile([HID, S * P], F32, tag="ffT", name="ffT")
                    nc.vector.tensor_copy(out=ffT[:, :Sg * P],
                                          in_=pffT[:, :Sg * P])
                    for s in range(Sg):
                        ptb = pp.tile([P, HID], F32, tag="ptb", name="ptb", bufs=1)
                        nc.tensor.transpose(
                            out=ptb[:], in_=ffT[:, s * P:(s + 1) * P],
                            identity=ident[:HID, :HID])
                        nc.vector.tensor_tensor(
                            out=xc[:, s * HID:(s + 1) * HID],
                            in0=xc[:, s * HID:(s + 1) * HID],
                            in1=ptb[:], op=OP.add)
                    nc.vector.tensor_tensor(
                        out=xc[:, :W].rearrange("p (s d) -> p s d", s=Sg),
                        in0=xc[:, :W].rearrange("p (s d) -> p s d", s=Sg),
                        in1=bf2x[l][:].unsqueeze(1).to_broadcast(
                            [P, Sg, HID]), op=OP.add)
                    _ln(nc, sp,
                        xc[:, :W].rearrange("p (s d) -> p s d", s=Sg),
                        Sg, HID, g2x[l], b2lx[l], "ln2", epsln)
                # fused output + GAT3 table
                for s in range(Sg):
                    jj = g0 + s
                    nc.sync.dma_start(
                        out=t_fused.ap()[jj * P:(jj + 1) * P, :],
                        in_=xc[:, s * HID:(s + 1) * HID])
                    ptt = pp.tile([HID, P], F32, tag="ptt", name="ptt", bufs=1)
                    nc.tensor.transpose(
                        out=ptt[:], in_=xc[:, s * HID:(s + 1) * HID],
                        identity=ident[:])
                    fT = sp.tile([HID, P], F32, tag="fT", name="fT")
                    nc.vector.tensor_copy(out=fT[:], in_=ptt[:])
                    p3 = pp.tile([P, OUT], F32, tag="p3", name="p3", bufs=1)
                    nc.tensor.matmul(p3[:], lhsT=fT[:], rhs=W3s[:],
                                     start=True, stop=True)
                    t3r = sp.tile([P, T3C], F32, tag="t3r", name="t3r")
                    nc.vector.tensor_copy(out=t3r[:, 0:OUT], in_=p3[:])
                    tmp3 = sp.tile([P, OUT], F32, tag="tmp3", name="tmp3")
                    nc.vector.tensor_tensor(out=tmp3[:], in0=t3r[:, :OUT],
                                            in1=a3sx[:], op=OP.mult)
                    nc.vector.reduce_sum(out=t3r[:, OUT:OUT + 1],
                                         in_=tmp3[:], axis=AX)
                    nc.vector.tensor_tensor(out=tmp3[:], in0=t3r[:, :OUT],
                                            in1=a3dx[:], op=OP.mult)
                    nc.vector.reduce_sum(out=adL3[:, jj:jj + 1],
                                         in_=tmp3[:], axis=AX)
                    nc.sync.dma_start(out=ag3_in[jj * P:(jj + 1) * P, :],
                                      in_=t3r[:])
        nc.gpsimd.collective_compute(
            "AllGather", OP.bypass, replica_groups=RG,
            ins=[ag3_in[:]], outs=[table3[:]])

        if DBG:
            nc.sync.dma_start(out=t_dt1.ap(), in_=table1[:])
            nc.sync.dma_start(out=t_dt2.ap(), in_=table2[:])
            nc.sync.dma_start(out=t_dt3.ap(), in_=table3[:])
            nc.sync.dma_start(out=t_drna.ap(), in_=rna[:])
            nc.sync.dma_start(out=t_dad1.ap(), in_=adL1[:])

        # ---------------- PHASE D: GAT3 head -> adt_pred --------------------
        with tc.tile_pool(name="pd", bufs=2) as sp, \
             tc.tile_pool(name="pds", bufs=2) as sps:
            for j in range(NBLK):
                nchk = (TPB[j] + CH3 - 1) // CH3
                S3 = sps.tile([P, T3C], F32, tag="S3", name="S3")
                for ci in range(nchk):
                    c0 = ci * CH3
                    cw = min(CH3, TPB[j] - c0)
                    G = sp.tile([P, CH3 * T3C], F32, tag="G3", name="G3")
                    M = sp.tile([P, CH3 * T3C], F32, tag="M3", name="M3")
                    Gv = G[:, :cw * T3C].rearrange("p (c w) -> p c w", w=T3C)
                    Mv = M[:, :cw * T3C].rearrange("p (c w) -> p c w", w=T3C)
                    for t in range(cw):
                        nc.gpsimd.indirect_dma_start(
                            out=G[:, t * T3C:(t + 1) * T3C], out_offset=None,
                            in_=table3[:],
                            in_offset=bass.IndirectOffsetOnAxis(
                                ap=srci[:, offs[j] + c0 + t:
                                        offs[j] + c0 + t + 1], axis=0))
                    nc.vector.tensor_tensor(
                        out=Mv[:, :, OUT:], in0=Gv[:, :, OUT:],
                        in1=adL3[:, j:j + 1].unsqueeze(1)
                        .to_broadcast([P, cw, 1]), op=OP.add)
                    nc.vector.tensor_tensor(
                        out=Mv[:, :, OUT:], in0=Mv[:, :, OUT:],
                        in1=lmsk[:, offs[j] + c0: offs[j] + c0 + cw]
                        .unsqueeze(2).to_broadcast([P, cw, 1]), op=OP.add)
                    zt = sp.tile([P, CH3], F32, tag="zt3", name="zt3")
                    zv = zt[:, :cw].unsqueeze(2)
                    nc.vector.tensor_scalar_mul(out=zv, in0=Mv[:, :, OUT:],
                                                scalar1=0.2)
                    nc.vector.tensor_tensor(out=Mv[:, :, OUT:],
                                            in0=Mv[:, :, OUT:], in1=zv,
                                            op=OP.max)
                    nc.scalar.activation(out=Mv[:, :, OUT:],
                                         in_=Mv[:, :, OUT:], func=ACTF.Exp)
                    nc.vector.tensor_tensor(
                        out=Mv[:, :, 0:OUT], in0=Gv[:, :, 0:OUT],
                        in1=Mv[:, :, OUT:OUT + 1].to_broadcast([P, cw, OUT]),
                        op=OP.mult)
                    Mt = M[:, :cw * T3C].rearrange(
                        "p (c w) -> p c w", w=T3C).transpose([0, 2, 1])
                    if ci == 0:
                        nc.vector.reduce_sum(out=S3[:], in_=Mt, axis=AX)
                    else:
                        St = sp.tile([P, T3C], F32, tag="St3", name="St3")
                        nc.vector.reduce_sum(out=St[:], in_=Mt, axis=AX)
                        nc.vector.tensor_add(out=S3[:], in0=S3[:], in1=St[:])
                rec = sps.tile([P, 1], F32, tag="rec3", name="rec3")
                nc.vector.tensor_scalar_add(out=rec[:], in0=S3[:, OUT:],
                                            scalar1=SM_EPS)
                nc.vector.reciprocal(out=rec[:], in_=rec[:])
                F = sps.tile([P, OUT], F32, tag="F3", name="F3")
                nc.vector.tensor_scalar_mul(out=F[:], in0=S3[:, 0:OUT],
                                            scalar1=rec[:, 0:1])
                nc.vector.tensor_tensor(out=F[:], in0=F[:], in1=b3x[:],
                                        op=OP.add)
                nc.sync.dma_start(out=t_adt.ap()[j * P:(j + 1) * P, :],
                                  in_=F[:])
        cp.release()
        dp.release()
    split_excess_waits(nc)
    return nc


def run(cfg, inputs, trace=False):
    in_maps, meta, post = host_prepare(cfg, inputs)
    nc = build_program(cfg, meta)
    res = bass_utils.run_bass_kernel_spmd(
        nc, in_maps, core_ids=list(range(cfg['NCORES'])), trace=trace)
    return post(res.results), res


def kernel(**inputs):
    cfg = make_cfg(N=50000, E=600000, IN_DIM=2000, HID=64, HEADS=4, OUT=25,
                   FF=2048, NLAYERS=2)
    (out, _res) = run(cfg, inputs)
    return out
